# revision 15
# baseline (speedup 1.0000x reference)
"""Differential attention + per-(batch,head) GroupNorm on 8 Trainium2 cores.

Math (reference):
    d = 128; Q1,Q2 = split(q); K1,K2 = split(k)
    w_m  = softmax(Q_m @ K_m^T / sqrt(d))           m in {1,2}
    lam  = exp(lam_q1.lam_k1) - exp(lam_q2.lam_k2) + 0.8
    out  = (w1 - lam*w2) @ v                         [B,H,S,D]
    y    = GroupNorm_{(b,h) over (S,D)}(out) * gamma + beta, scaled by 0.2

Strategy:
  * Shard the 32 (b,h) pairs as 4 per core (GroupNorm is per (b,h): no
    cross-core reductions).
  * alpha_1 = 1, alpha_2 = -lam are computed on host.  Output of GroupNorm is
    invariant under positive scaling of `out`, so we rescale alphas by
    1/max|alpha| and drop any attention branch whose relative weight is
    below 1e-9 (with the reference inputs lam ~ 3.7e16, so branch 1
    contributes ~2.7e-17 relatively - far below fp32 resolution).
  * Per (b,h): K^T/Q^T via PE transposes (bf16); S^T chunks [t,s] = K_T.T @
    Q_T with bf16 matmuls (N=512); exp on ACT straight out of PSUM
    (scale=1/sqrt(d), shift -2 folded into the activation) writing fp16 E^T;
    PV uses E^T chunks as stationary against [V | 1] so the softmax
    denominator Z rides along as matmul column 128; normalize + GroupNorm
    stats on DVE (batched whole-tile reduces); cross-partition stat reduction
    via a ones-matmul; rstd = 1/sqrt(var+eps) via DVE bit-trick + Newton so
    Exp stays the only ACT table (single table load).
"""

import math
import sys

import numpy as np

if "/opt/trn_rl_repo" not in sys.path:
    sys.path.insert(0, "/opt/trn_rl_repo")

B, H, S, D = 2, 16, 2048, 128
N_CORES = 8
BH_PER_CORE = B * H // N_CORES  # 4
LAMBDA_INIT = 0.8
GN_EPS = 1e-5

SCALE = 1.0 / math.sqrt(D)
EXP_SHIFT = -2.0  # exp(s*SCALE + EXP_SHIFT): keeps E in fp16 range; cancels in E/Z
N_CHUNK = S // 128  # 16 key chunks per (b,h)
SIGMA = 512  # query super-tile width
N_SIG = S // SIGMA  # 4
QT_PER_SIG = SIGMA // 128  # 4
# (chunk_start, n_chunks) groups per sigma; tiles sized [128, n, 512] so the
# qk PSUM pool slot stays at 3 banks (2 slots + small pool = 8 banks total)
CHUNK_GROUPS = [(0, 3), (3, 3), (6, 3), (9, 3), (12, 2), (14, 2)]

_BUILD_CACHE: dict = {}
LAST_RESULTS = None  # BassKernelResults of the most recent run (for test.py)


def _build(active, alphas, eps_eff, nbh=BH_PER_CORE):
    """Build + compile the per-core Bass module.

    active: tuple of matrix indices (subset of (0, 1)) that contribute.
    alphas: rescaled combine weights for the active matrices (|.| <= 1).
    eps_eff: GroupNorm eps after alpha rescaling.
    """
    from contextlib import ExitStack

    import concourse.bacc as bacc
    import concourse.mybir as mybir
    import concourse.tile as tile
    from concourse.masks import make_identity

    f32 = mybir.dt.float32
    f32r = mybir.dt.float32r
    bf16 = mybir.dt.bfloat16
    f16 = mybir.dt.float16
    AF = mybir.ActivationFunctionType
    ALU = mybir.AluOpType

    nc = bacc.Bacc("TRN2", target_bir_lowering=False, debug=False)

    qk_in = {}
    for m in active:
        qk_in[m] = (
            nc.dram_tensor(f"q_m{m}", [nbh, S, D], f32, kind="ExternalInput").ap(),
            nc.dram_tensor(f"k_m{m}", [nbh, S, D], f32, kind="ExternalInput").ap(),
        )
    v_in = nc.dram_tensor("v_in", [nbh, S, D], f32, kind="ExternalInput").ap()
    gw_in = nc.dram_tensor("gw", [nbh, D], f32, kind="ExternalInput").ap()
    gb_in = nc.dram_tensor("gb", [nbh, D], f32, kind="ExternalInput").ap()
    out_t = nc.dram_tensor("out", [nbh, S, D], f32, kind="ExternalOutput").ap()

    n_act = len(active)

    with tile.TileContext(nc) as tc, ExitStack() as ex:
        consts = ex.enter_context(tc.tile_pool(name="consts", bufs=1))
        raw = ex.enter_context(tc.tile_pool(name="raw", bufs=2))
        tsp = ex.enter_context(tc.tile_pool(name="tsp", bufs=2))  # Q^T/K^T
        vbp = ex.enter_context(tc.tile_pool(name="vbp", bufs=2))
        ep = ex.enter_context(tc.tile_pool(name="ep", bufs=3))  # E^T per sigma
        resp = ex.enter_context(tc.tile_pool(name="resp", bufs=2))
        gwp = ex.enter_context(tc.tile_pool(name="gwp", bufs=2))
        statp = ex.enter_context(tc.tile_pool(name="statp", bufs=2))
        smallv = ex.enter_context(tc.tile_pool(name="smallv", bufs=4))
        qkp = ex.enter_context(tc.tile_pool(name="qkp", bufs=2, space="PSUM"))
        smallp = ex.enter_context(tc.tile_pool(name="smallp", bufs=2, space="PSUM"))

        identity = consts.tile([128, 128], f32)
        make_identity(nc, identity)
        ones128 = consts.tile([128, 128], f32)
        nc.gpsimd.memset(ones128, 1.0)
        exp_bias = consts.tile([128, 1], f32)
        nc.gpsimd.memset(exp_bias, EXP_SHIFT)
        eps_tile = consts.tile([128, 1], f32)
        nc.gpsimd.memset(eps_tile, float(eps_eff))

        for bh in range(nbh):
            # ---- load + transpose Q/K, load V -------------------------------
            tq = {}
            tk = {}
            for m in active:
                q_ap, k_ap = qk_in[m]
                qraw = raw.tile([128, N_CHUNK, D], f32, tag=f"qraw{m}")
                kraw = raw.tile([128, N_CHUNK, D], f32, tag=f"kraw{m}")
                nc.sync.dma_start(
                    out=qraw, in_=q_ap[bh].rearrange("(c p) d -> p c d", p=128)
                )
                nc.sync.dma_start(
                    out=kraw, in_=k_ap[bh].rearrange("(c p) d -> p c d", p=128)
                )
                # bf16 QK path: the PSUM->SBUF copy converts f32->bf16
                qT = tsp.tile([128, S], bf16, tag=f"qT{m}")  # [d, s]
                kT = tsp.tile([128, S], bf16, tag=f"kT{m}")  # [d, t]
                for src, dst in ((qraw, qT), (kraw, kT)):
                    for g in range(N_CHUNK // 4):
                        tp = smallp.tile([128, 4, 128], f32, tag="sm")
                        for j in range(4):
                            nc.tensor.transpose(tp[:, j], src[:, 4 * g + j], identity)
                        nc.vector.tensor_copy(
                            out=dst[:, 4 * g * 128 : 4 * (g + 1) * 128], in_=tp
                        )
                tq[m] = qT
                tk[m] = kT

            vraw = raw.tile([128, N_CHUNK, D], f32, tag="vraw")
            nc.sync.dma_start(
                out=vraw, in_=v_in[bh].rearrange("(c p) d -> p c d", p=128)
            )
            vb = vbp.tile([128, N_CHUNK, D + 1], f16)  # [t, (chunk, d | 1)]
            nc.vector.tensor_copy(out=vb[:, :, 0:D], in_=vraw)
            nc.vector.memset(vb[:, :, D : D + 1], 1.0)

            gwb = gwp.tile([128, D], f32, tag="gw")
            gbb = gwp.tile([128, D], f32, tag="gb")
            nc.sync.dma_start(out=gwb, in_=gw_in[bh].partition_broadcast(128))
            nc.sync.dma_start(out=gbb, in_=gb_in[bh].partition_broadcast(128))

            stats = statp.tile([128, 2, N_CHUNK], f32, tag="stats")
            res = resp.tile([128, N_CHUNK, D], f32, tag="res")
            sqbig = resp.tile([128, N_CHUNK, D], f32, tag="sqbig", bufs=1)

            # ---- attention over query super-tiles ---------------------------
            for sg in range(N_SIG):
                eT = {}
                for m in active:
                    emt = ep.tile([128, N_CHUNK, SIGMA], f16, tag=f"eT{m}")
                    for c0, ncnk in CHUNK_GROUPS:
                        sp = qkp.tile([128, 3, SIGMA], f32, tag="qk")
                        for j in range(ncnk):
                            c = c0 + j
                            nc.tensor.matmul(
                                sp[:, j],
                                tk[m][:, c * 128 : (c + 1) * 128],
                                tq[m][:, sg * SIGMA : (sg + 1) * SIGMA],
                                start=True,
                                stop=True,
                            )
                        nc.scalar.activation(
                            out=emt[:, c0 : c0 + ncnk],
                            in_=sp[:, 0:ncnk],
                            func=AF.Exp,
                            bias=exp_bias,
                            scale=SCALE,
                        )
                    eT[m] = emt

                for j in range(QT_PER_SIG):
                    qt = sg * QT_PER_SIG + j
                    sl = slice(j * 128, (j + 1) * 128)
                    for mi, m in enumerate(active):
                        op = smallp.tile([128, D + 1], f32, tag="sm")
                        for c in range(N_CHUNK):
                            nc.tensor.matmul(
                                op,
                                eT[m][:, c, sl],
                                vb[:, c],
                                start=(c == 0),
                                stop=(c == N_CHUNK - 1),
                            )
                        rz = smallv.tile([128, 1], f32, tag="rz")
                        nc.vector.reciprocal(rz, op[:, D : D + 1])
                        if alphas[mi] != 1.0:
                            nc.vector.tensor_scalar_mul(rz, rz, float(alphas[mi]))
                        if mi == 0:
                            nc.vector.tensor_scalar(
                                out=res[:, qt],
                                in0=op[:, 0:D],
                                scalar1=rz,
                                scalar2=None,
                                op0=ALU.mult,
                            )
                        else:
                            tmp = smallv.tile([128, D], f32, tag="tmp")
                            nc.vector.tensor_scalar(
                                out=tmp,
                                in0=op[:, 0:D],
                                scalar1=rz,
                                scalar2=None,
                                op0=ALU.mult,
                            )
                            nc.vector.tensor_tensor(
                                out=res[:, qt], in0=res[:, qt], in1=tmp, op=ALU.add
                            )

            # ---- GroupNorm over (S, D) for this (b,h) -----------------------
            # batched row-stats over the whole res tile (fewer DVE instructions)
            nc.vector.tensor_reduce(
                out=stats[:, 0, :], in_=res, axis=mybir.AxisListType.X, op=ALU.add
            )
            nc.vector.tensor_tensor(out=sqbig, in0=res, in1=res, op=ALU.mult)
            nc.vector.tensor_reduce(
                out=stats[:, 1, :], in_=sqbig, axis=mybir.AxisListType.X, op=ALU.add
            )
            gp = smallp.tile([128, 2, N_CHUNK], f32, tag="sm")
            nc.tensor.matmul(gp, ones128, stats, start=True, stop=True)
            tot = smallv.tile([128, 1], f32, tag="tot")
            sqt = smallv.tile([128, 1], f32, tag="sqt")
            nc.vector.tensor_reduce(
                out=tot, in_=gp[:, 0], axis=mybir.AxisListType.X, op=ALU.add
            )
            nc.vector.tensor_reduce(
                out=sqt, in_=gp[:, 1], axis=mybir.AxisListType.X, op=ALU.add
            )
            inv_n = 1.0 / float(S * D)
            mean = smallv.tile([128, 1], f32, tag="mean")
            var = smallv.tile([128, 1], f32, tag="var")
            nc.vector.tensor_scalar_mul(mean, tot, inv_n)
            # var = E[x^2] - mean^2
            nc.vector.tensor_scalar_mul(var, sqt, inv_n)
            msq = smallv.tile([128, 1], f32, tag="msq")
            nc.vector.tensor_tensor(out=msq, in0=mean, in1=mean, op=ALU.mult)
            nc.vector.tensor_tensor(out=var, in0=var, in1=msq, op=ALU.subtract)
            # rstd = 1/sqrt(var + eps) on DVE (bit-trick + 3 Newton steps)
            # avoids Ln/Sqrt ACT table loads (exp stays the only ACT table)
            nc.vector.tensor_scalar_add(var, var, float(eps_eff))
            rstd = smallv.tile([128, 1], f32, tag="rstd")
            hv = smallv.tile([128, 1], f32, tag="hv")
            t0 = smallv.tile([128, 1], f32, tag="t0")
            nc.vector.tensor_scalar_mul(hv, var, 0.5)
            ri = rstd.bitcast(mybir.dt.int32)
            nc.vector.tensor_scalar(
                out=ri, in0=var.bitcast(mybir.dt.int32), scalar1=1, scalar2=None,
                op0=ALU.logical_shift_right,
            )
            nc.vector.tensor_scalar(
                out=ri, in0=ri, scalar1=-1, scalar2=0x5F3759DF, op0=ALU.mult, op1=ALU.add
            )
            for _ in range(3):
                nc.vector.tensor_tensor(out=t0, in0=rstd, in1=rstd, op=ALU.mult)
                nc.vector.tensor_tensor(out=t0, in0=t0, in1=hv, op=ALU.mult)
                nc.vector.tensor_scalar(
                    out=t0, in0=t0, scalar1=-1.0, scalar2=1.5, op0=ALU.mult, op1=ALU.add
                )
                nc.vector.tensor_tensor(out=rstd, in0=rstd, in1=t0, op=ALU.mult)
            # A = gw * rstd ; Bt = gb - A * mean   (y = res*A + Bt)
            A = statp.tile([128, D], f32, tag="A")
            Bt = statp.tile([128, D], f32, tag="Bt")
            nc.vector.tensor_scalar(
                out=A, in0=gwb, scalar1=rstd, scalar2=None, op0=ALU.mult
            )
            nc.vector.tensor_scalar(
                out=Bt, in0=A, scalar1=mean, scalar2=None, op0=ALU.mult
            )
            nc.vector.tensor_tensor(out=Bt, in0=gbb, in1=Bt, op=ALU.subtract)
            a_b = A.unsqueeze(1).broadcast_to((128, N_CHUNK, D))
            bt_b = Bt.unsqueeze(1).broadcast_to((128, N_CHUNK, D))
            nc.vector.tensor_tensor(out=res, in0=res, in1=a_b, op=ALU.mult)
            nc.vector.tensor_tensor(out=res, in0=res, in1=bt_b, op=ALU.add)
            nc.sync.dma_start(
                out=out_t[bh].rearrange("(c p) d -> p c d", p=128), in_=res
            )

    nc.compile()
    return nc


def kernel(q, k, v, lam_q1, lam_q2, lam_k1, lam_k2, gn_weight, gn_bias):
    global LAST_RESULTS
    import os

    os.environ.setdefault("MYCRO_LOCAL_CACHE", "1")
    from concourse.bass_utils import run_bass_kernel_spmd

    q = np.asarray(q, dtype=np.float32)
    k = np.asarray(k, dtype=np.float32)
    v = np.asarray(v, dtype=np.float32)

    d1 = float(np.dot(np.asarray(lam_q1, np.float64), np.asarray(lam_k1, np.float64)))
    d2 = float(np.dot(np.asarray(lam_q2, np.float64), np.asarray(lam_k2, np.float64)))
    lam = math.exp(d1) - math.exp(d2) + LAMBDA_INIT
    alpha = [1.0, -lam]
    amax = max(abs(a) for a in alpha)
    active = tuple(m for m in (0, 1) if abs(alpha[m]) / amax > 1e-9)
    alphas = tuple(alpha[m] / amax for m in active)
    eps_eff = GN_EPS / (amax * amax)

    key = (active, alphas, round(math.log10(max(eps_eff, 1e-300)), 3))
    if key not in _BUILD_CACHE:
        _BUILD_CACHE[key] = _build(active, alphas, eps_eff)
    nc = _BUILD_CACHE[key]

    qf = q.reshape(B * H, S, 2 * D)
    kf = k.reshape(B * H, S, 2 * D)
    vf = v.reshape(B * H, S, D)
    gw = (np.asarray(gn_weight, np.float32) * (1.0 - LAMBDA_INIT)).reshape(H, D)
    gb = (np.asarray(gn_bias, np.float32) * (1.0 - LAMBDA_INIT)).reshape(H, D)

    in_maps = []
    for core in range(N_CORES):
        s0 = core * BH_PER_CORE
        bhs = range(s0, s0 + BH_PER_CORE)
        im = {
            "v_in": np.ascontiguousarray(vf[s0 : s0 + BH_PER_CORE]),
            "gw": np.ascontiguousarray(np.stack([gw[bh % H] for bh in bhs])),
            "gb": np.ascontiguousarray(np.stack([gb[bh % H] for bh in bhs])),
        }
        for m in active:
            im[f"q_m{m}"] = np.ascontiguousarray(
                qf[s0 : s0 + BH_PER_CORE, :, m * D : (m + 1) * D]
            )
            im[f"k_m{m}"] = np.ascontiguousarray(
                kf[s0 : s0 + BH_PER_CORE, :, m * D : (m + 1) * D]
            )
        in_maps.append(im)

    # one retry: a previously wedged device surfaces as a transient
    # JaxRuntimeError (NRT_EXEC_UNIT_UNRECOVERABLE) and recovers on re-dispatch
    try:
        LAST_RESULTS = run_bass_kernel_spmd(nc, in_maps, core_ids=list(range(N_CORES)))
    except Exception:
        import time

        time.sleep(5)
        LAST_RESULTS = run_bass_kernel_spmd(nc, in_maps, core_ids=list(range(N_CORES)))
    out = np.concatenate([r["out"] for r in LAST_RESULTS.results], axis=0)
    return out.reshape(B, H, S, D).astype(np.float32)


# revision 16
# speedup vs baseline: 1.0168x; 1.0168x over previous
"""Differential attention + per-(batch,head) GroupNorm on 8 Trainium2 cores.

Math (reference):
    d = 128; Q1,Q2 = split(q); K1,K2 = split(k)
    w_m  = softmax(Q_m @ K_m^T / sqrt(d))           m in {1,2}
    lam  = exp(lam_q1.lam_k1) - exp(lam_q2.lam_k2) + 0.8
    out  = (w1 - lam*w2) @ v                         [B,H,S,D]
    y    = GroupNorm_{(b,h) over (S,D)}(out) * gamma + beta, scaled by 0.2

Strategy:
  * Shard the 32 (b,h) pairs as 4 per core (GroupNorm is per (b,h): no
    cross-core reductions).
  * alpha_1 = 1, alpha_2 = -lam are computed on host.  Output of GroupNorm is
    invariant under positive scaling of `out`, so we rescale alphas by
    1/max|alpha| and drop any attention branch whose relative weight is
    below 1e-9 (with the reference inputs lam ~ 3.7e16, so branch 1
    contributes ~2.7e-17 relatively - far below fp32 resolution).
  * Per (b,h): K^T/Q^T via PE transposes (bf16); S^T chunks [t,s] = K_T.T @
    Q_T with bf16 matmuls (N=512); exp on ACT straight out of PSUM
    (scale=1/sqrt(d), shift -2 folded into the activation) writing fp16 E^T;
    PV uses E^T chunks as stationary against [V | 1] so the softmax
    denominator Z rides along as matmul column 128; normalize + GroupNorm
    stats on DVE (batched whole-tile reduces); cross-partition stat reduction
    via a ones-matmul; rstd = 1/sqrt(var+eps) via DVE bit-trick + Newton so
    Exp stays the only ACT table (single table load).
"""

import math
import sys

import numpy as np

if "/opt/trn_rl_repo" not in sys.path:
    sys.path.insert(0, "/opt/trn_rl_repo")

B, H, S, D = 2, 16, 2048, 128
N_CORES = 8
BH_PER_CORE = B * H // N_CORES  # 4
LAMBDA_INIT = 0.8
GN_EPS = 1e-5

SCALE = 1.0 / math.sqrt(D)
EXP_SHIFT = -2.0  # exp(s*SCALE + EXP_SHIFT): keeps E in fp16 range; cancels in E/Z
N_CHUNK = S // 128  # 16 key chunks per (b,h)
SIGMA = 512  # query super-tile width
N_SIG = S // SIGMA  # 4
QT_PER_SIG = SIGMA // 128  # 4
# (chunk_start, n_chunks) groups per sigma; tiles sized [128, n, 512] so the
# qk PSUM pool slot stays at 3 banks (2 slots + small pool = 8 banks total)
CHUNK_GROUPS = [(0, 3), (3, 3), (6, 3), (9, 3), (12, 2), (14, 2)]

_BUILD_CACHE: dict = {}
LAST_RESULTS = None  # BassKernelResults of the most recent run (for test.py)


def _build(active, alphas, eps_eff, nbh=BH_PER_CORE):
    """Build + compile the per-core Bass module.

    active: tuple of matrix indices (subset of (0, 1)) that contribute.
    alphas: rescaled combine weights for the active matrices (|.| <= 1).
    eps_eff: GroupNorm eps after alpha rescaling.
    """
    from contextlib import ExitStack

    import concourse.bacc as bacc
    import concourse.mybir as mybir
    import concourse.tile as tile
    from concourse.masks import make_identity

    f32 = mybir.dt.float32
    f32r = mybir.dt.float32r
    bf16 = mybir.dt.bfloat16
    f16 = mybir.dt.float16
    AF = mybir.ActivationFunctionType
    ALU = mybir.AluOpType

    nc = bacc.Bacc("TRN2", target_bir_lowering=False, debug=False)

    qk_in = {}
    for m in active:
        qk_in[m] = (
            nc.dram_tensor(f"q_m{m}", [nbh, S, D], f32, kind="ExternalInput").ap(),
            nc.dram_tensor(f"k_m{m}", [nbh, S, D], f32, kind="ExternalInput").ap(),
        )
    v_in = nc.dram_tensor("v_in", [nbh, S, D], f32, kind="ExternalInput").ap()
    gw_in = nc.dram_tensor("gw", [nbh, D], f32, kind="ExternalInput").ap()
    gb_in = nc.dram_tensor("gb", [nbh, D], f32, kind="ExternalInput").ap()
    out_t = nc.dram_tensor("out", [nbh, S, D], f32, kind="ExternalOutput").ap()

    n_act = len(active)

    with tile.TileContext(nc) as tc, ExitStack() as ex:
        consts = ex.enter_context(tc.tile_pool(name="consts", bufs=1))
        raw = ex.enter_context(tc.tile_pool(name="raw", bufs=2))
        tsp = ex.enter_context(tc.tile_pool(name="tsp", bufs=2))  # Q^T/K^T
        vbp = ex.enter_context(tc.tile_pool(name="vbp", bufs=2))
        ep = ex.enter_context(tc.tile_pool(name="ep", bufs=2))  # E^T per sigma
        resp = ex.enter_context(tc.tile_pool(name="resp", bufs=2))
        gwp = ex.enter_context(tc.tile_pool(name="gwp", bufs=2))
        statp = ex.enter_context(tc.tile_pool(name="statp", bufs=2))
        smallv = ex.enter_context(tc.tile_pool(name="smallv", bufs=4))
        qkp = ex.enter_context(tc.tile_pool(name="qkp", bufs=2, space="PSUM"))
        smallp = ex.enter_context(tc.tile_pool(name="smallp", bufs=2, space="PSUM"))

        identity = consts.tile([128, 128], f32)
        make_identity(nc, identity)
        ones128 = consts.tile([128, 128], f32)
        nc.gpsimd.memset(ones128, 1.0)
        exp_bias = consts.tile([128, 1], f32)
        nc.gpsimd.memset(exp_bias, EXP_SHIFT)
        eps_tile = consts.tile([128, 1], f32)
        nc.gpsimd.memset(eps_tile, float(eps_eff))

        for bh in range(nbh):
            # ---- load + transpose Q/K, load V -------------------------------
            tq = {}
            tk = {}
            for m in active:
                q_ap, k_ap = qk_in[m]
                qraw = raw.tile([128, N_CHUNK, D], f32, tag=f"qraw{m}")
                kraw = raw.tile([128, N_CHUNK, D], f32, tag=f"kraw{m}")
                nc.sync.dma_start(
                    out=qraw, in_=q_ap[bh].rearrange("(c p) d -> p c d", p=128)
                )
                nc.sync.dma_start(
                    out=kraw, in_=k_ap[bh].rearrange("(c p) d -> p c d", p=128)
                )
                # bf16 QK path: the PSUM->SBUF copy converts f32->bf16
                qT = tsp.tile([128, S], bf16, tag=f"qT{m}")  # [d, s]
                kT = tsp.tile([128, S], bf16, tag=f"kT{m}")  # [d, t]
                for src, dst in ((qraw, qT), (kraw, kT)):
                    for g in range(N_CHUNK // 4):
                        tp = smallp.tile([128, 4, 128], f32, tag="sm")
                        for j in range(4):
                            nc.tensor.transpose(tp[:, j], src[:, 4 * g + j], identity)
                        nc.vector.tensor_copy(
                            out=dst[:, 4 * g * 128 : 4 * (g + 1) * 128], in_=tp
                        )
                tq[m] = qT
                tk[m] = kT

            vraw = raw.tile([128, N_CHUNK, D], f32, tag="vraw")
            nc.sync.dma_start(
                out=vraw, in_=v_in[bh].rearrange("(c p) d -> p c d", p=128)
            )
            vb = vbp.tile([128, N_CHUNK, D + 1], f16)  # [t, (chunk, d | 1)]
            nc.vector.tensor_copy(out=vb[:, :, 0:D], in_=vraw)
            nc.vector.memset(vb[:, :, D : D + 1], 1.0)

            gwb = gwp.tile([128, D], f32, tag="gw")
            gbb = gwp.tile([128, D], f32, tag="gb")
            nc.sync.dma_start(out=gwb, in_=gw_in[bh].partition_broadcast(128))
            nc.sync.dma_start(out=gbb, in_=gb_in[bh].partition_broadcast(128))

            stats = statp.tile([128, 2, N_CHUNK], f32, tag="stats")
            res = resp.tile([128, N_CHUNK, D], f32, tag="res")
            sqbig = resp.tile([128, N_CHUNK, D], f32, tag="sqbig", bufs=1)

            # ---- attention over query super-tiles ---------------------------
            for sg in range(N_SIG):
                eT = {}
                for m in active:
                    emt = ep.tile([128, N_CHUNK, SIGMA], f16, tag=f"eT{m}")
                    for c0, ncnk in CHUNK_GROUPS:
                        sp = qkp.tile([128, 3, SIGMA], f32, tag="qk")
                        for j in range(ncnk):
                            c = c0 + j
                            nc.tensor.matmul(
                                sp[:, j],
                                tk[m][:, c * 128 : (c + 1) * 128],
                                tq[m][:, sg * SIGMA : (sg + 1) * SIGMA],
                                start=True,
                                stop=True,
                            )
                        nc.scalar.activation(
                            out=emt[:, c0 : c0 + ncnk],
                            in_=sp[:, 0:ncnk],
                            func=AF.Exp,
                            bias=exp_bias,
                            scale=SCALE,
                        )
                    eT[m] = emt

                for j in range(QT_PER_SIG):
                    qt = sg * QT_PER_SIG + j
                    sl = slice(j * 128, (j + 1) * 128)
                    for mi, m in enumerate(active):
                        op = smallp.tile([128, D + 1], f32, tag="sm")
                        for c in range(N_CHUNK):
                            nc.tensor.matmul(
                                op,
                                eT[m][:, c, sl],
                                vb[:, c],
                                start=(c == 0),
                                stop=(c == N_CHUNK - 1),
                            )
                        rz = smallv.tile([128, 1], f32, tag="rz")
                        nc.vector.reciprocal(rz, op[:, D : D + 1])
                        if alphas[mi] != 1.0:
                            nc.vector.tensor_scalar_mul(rz, rz, float(alphas[mi]))
                        if mi == 0:
                            nc.vector.tensor_scalar(
                                out=res[:, qt],
                                in0=op[:, 0:D],
                                scalar1=rz,
                                scalar2=None,
                                op0=ALU.mult,
                            )
                        else:
                            tmp = smallv.tile([128, D], f32, tag="tmp")
                            nc.vector.tensor_scalar(
                                out=tmp,
                                in0=op[:, 0:D],
                                scalar1=rz,
                                scalar2=None,
                                op0=ALU.mult,
                            )
                            nc.vector.tensor_tensor(
                                out=res[:, qt], in0=res[:, qt], in1=tmp, op=ALU.add
                            )

            # ---- GroupNorm over (S, D) for this (b,h) -----------------------
            # batched row-stats over the whole res tile (fewer DVE instructions)
            nc.vector.tensor_reduce(
                out=stats[:, 0, :], in_=res, axis=mybir.AxisListType.X, op=ALU.add
            )
            nc.vector.tensor_tensor(out=sqbig, in0=res, in1=res, op=ALU.mult)
            nc.vector.tensor_reduce(
                out=stats[:, 1, :], in_=sqbig, axis=mybir.AxisListType.X, op=ALU.add
            )
            gp = smallp.tile([128, 2, N_CHUNK], f32, tag="sm")
            nc.tensor.matmul(gp, ones128, stats, start=True, stop=True)
            tot = smallv.tile([128, 1], f32, tag="tot")
            sqt = smallv.tile([128, 1], f32, tag="sqt")
            nc.vector.tensor_reduce(
                out=tot, in_=gp[:, 0], axis=mybir.AxisListType.X, op=ALU.add
            )
            nc.vector.tensor_reduce(
                out=sqt, in_=gp[:, 1], axis=mybir.AxisListType.X, op=ALU.add
            )
            inv_n = 1.0 / float(S * D)
            mean = smallv.tile([128, 1], f32, tag="mean")
            var = smallv.tile([128, 1], f32, tag="var")
            nc.vector.tensor_scalar_mul(mean, tot, inv_n)
            # var = E[x^2] - mean^2
            nc.vector.tensor_scalar_mul(var, sqt, inv_n)
            msq = smallv.tile([128, 1], f32, tag="msq")
            nc.vector.tensor_tensor(out=msq, in0=mean, in1=mean, op=ALU.mult)
            nc.vector.tensor_tensor(out=var, in0=var, in1=msq, op=ALU.subtract)
            # rstd = 1/sqrt(var + eps) on DVE (bit-trick + 3 Newton steps)
            # avoids Ln/Sqrt ACT table loads (exp stays the only ACT table)
            nc.vector.tensor_scalar_add(var, var, float(eps_eff))
            rstd = smallv.tile([128, 1], f32, tag="rstd")
            hv = smallv.tile([128, 1], f32, tag="hv")
            t0 = smallv.tile([128, 1], f32, tag="t0")
            nc.vector.tensor_scalar_mul(hv, var, 0.5)
            ri = rstd.bitcast(mybir.dt.int32)
            nc.vector.tensor_scalar(
                out=ri, in0=var.bitcast(mybir.dt.int32), scalar1=1, scalar2=None,
                op0=ALU.logical_shift_right,
            )
            nc.vector.tensor_scalar(
                out=ri, in0=ri, scalar1=-1, scalar2=0x5F3759DF, op0=ALU.mult, op1=ALU.add
            )
            for _ in range(3):
                nc.vector.tensor_tensor(out=t0, in0=rstd, in1=rstd, op=ALU.mult)
                nc.vector.tensor_tensor(out=t0, in0=t0, in1=hv, op=ALU.mult)
                nc.vector.tensor_scalar(
                    out=t0, in0=t0, scalar1=-1.0, scalar2=1.5, op0=ALU.mult, op1=ALU.add
                )
                nc.vector.tensor_tensor(out=rstd, in0=rstd, in1=t0, op=ALU.mult)
            # A = gw * rstd ; Bt = gb - A * mean   (y = res*A + Bt)
            A = statp.tile([128, D], f32, tag="A")
            Bt = statp.tile([128, D], f32, tag="Bt")
            nc.vector.tensor_scalar(
                out=A, in0=gwb, scalar1=rstd, scalar2=None, op0=ALU.mult
            )
            nc.vector.tensor_scalar(
                out=Bt, in0=A, scalar1=mean, scalar2=None, op0=ALU.mult
            )
            nc.vector.tensor_tensor(out=Bt, in0=gbb, in1=Bt, op=ALU.subtract)
            a_b = A.unsqueeze(1).broadcast_to((128, N_CHUNK, D))
            bt_b = Bt.unsqueeze(1).broadcast_to((128, N_CHUNK, D))
            nc.vector.tensor_tensor(out=res, in0=res, in1=a_b, op=ALU.mult)
            nc.vector.tensor_tensor(out=res, in0=res, in1=bt_b, op=ALU.add)
            nc.sync.dma_start(
                out=out_t[bh].rearrange("(c p) d -> p c d", p=128), in_=res
            )

    nc.compile()
    return nc


def kernel(q, k, v, lam_q1, lam_q2, lam_k1, lam_k2, gn_weight, gn_bias):
    global LAST_RESULTS
    import os

    os.environ.setdefault("MYCRO_LOCAL_CACHE", "1")
    from concourse.bass_utils import run_bass_kernel_spmd

    q = np.asarray(q, dtype=np.float32)
    k = np.asarray(k, dtype=np.float32)
    v = np.asarray(v, dtype=np.float32)

    d1 = float(np.dot(np.asarray(lam_q1, np.float64), np.asarray(lam_k1, np.float64)))
    d2 = float(np.dot(np.asarray(lam_q2, np.float64), np.asarray(lam_k2, np.float64)))
    lam = math.exp(d1) - math.exp(d2) + LAMBDA_INIT
    alpha = [1.0, -lam]
    amax = max(abs(a) for a in alpha)
    active = tuple(m for m in (0, 1) if abs(alpha[m]) / amax > 1e-9)
    alphas = tuple(alpha[m] / amax for m in active)
    eps_eff = GN_EPS / (amax * amax)

    key = (active, alphas, round(math.log10(max(eps_eff, 1e-300)), 3))
    if key not in _BUILD_CACHE:
        _BUILD_CACHE[key] = _build(active, alphas, eps_eff)
    nc = _BUILD_CACHE[key]

    qf = q.reshape(B * H, S, 2 * D)
    kf = k.reshape(B * H, S, 2 * D)
    vf = v.reshape(B * H, S, D)
    gw = (np.asarray(gn_weight, np.float32) * (1.0 - LAMBDA_INIT)).reshape(H, D)
    gb = (np.asarray(gn_bias, np.float32) * (1.0 - LAMBDA_INIT)).reshape(H, D)

    in_maps = []
    for core in range(N_CORES):
        s0 = core * BH_PER_CORE
        bhs = range(s0, s0 + BH_PER_CORE)
        im = {
            "v_in": np.ascontiguousarray(vf[s0 : s0 + BH_PER_CORE]),
            "gw": np.ascontiguousarray(np.stack([gw[bh % H] for bh in bhs])),
            "gb": np.ascontiguousarray(np.stack([gb[bh % H] for bh in bhs])),
        }
        for m in active:
            im[f"q_m{m}"] = np.ascontiguousarray(
                qf[s0 : s0 + BH_PER_CORE, :, m * D : (m + 1) * D]
            )
            im[f"k_m{m}"] = np.ascontiguousarray(
                kf[s0 : s0 + BH_PER_CORE, :, m * D : (m + 1) * D]
            )
        in_maps.append(im)

    # one retry: a previously wedged device surfaces as a transient
    # JaxRuntimeError (NRT_EXEC_UNIT_UNRECOVERABLE) and recovers on re-dispatch
    try:
        LAST_RESULTS = run_bass_kernel_spmd(nc, in_maps, core_ids=list(range(N_CORES)))
    except Exception:
        import time

        time.sleep(5)
        LAST_RESULTS = run_bass_kernel_spmd(nc, in_maps, core_ids=list(range(N_CORES)))
    out = np.concatenate([r["out"] for r in LAST_RESULTS.results], axis=0)
    return out.reshape(B, H, S, D).astype(np.float32)
